# revision 15
# baseline (speedup 1.0000x reference)
"""GroupedEmbedding lookup kernel for 8 Trainium2 NeuronCores.

Sharding: table-wise, 2 tables per core. Each core holds a [2*R, D] weight
slab and processes the matching 262144 lookups; its output is a contiguous
block of the final [T*L, D] output.

Device kernel v2 (bulk-descriptor gather/scatter via GPSIMD mlp library):

The baseline used one indirect_dma_start per 128 lookups; SWDGE descriptor
generation costs ~1 us *per instruction* (994 ns fixed + 0.34 ns/descriptor),
so 2048 instructions serialized on the Pool engine at ~2.9 ms. dma_gather
(InstDMAGatherAnt) generates thousands of descriptors in ONE instruction,
but takes int16 indices (<= 32767) while the per-core slab has 400000 rows.

Per 32768-lookup window:
  1. Host buckets the window's lookups by 32768-row block (13 blocks) and
     emits per-block int16 local indices (padded to a fixed cap with idx 0)
     plus int16 scatter codes that undo the bucket permutation.
  2. 13x dma_gather: block rows -> staging SBUF (bucket order).
  3. 1x dma_scatter_add (SBUF parity mode) staging -> own/peer buffers:
     scatter code v = (i & 255)*128 + (i >> 8) routes window position i to
     own/peer[partition i>>8, group (i&255)>>1]; padding goes to v=0 (trash;
     position 0 of each window is patched on the host afterwards).
     own/peer are pre-zeroed by DVE (scatter is an ADD).
  4. DVE interleaves own/peer -> merged bf16 [128, 256*D] (position-ordered,
     contiguous per partition).
  5. gpsimd cast-store (bf16 -> f32) merged -> output window (64 KB/partition
     contiguous runs).

HBM traffic/core ~= 73 MB gather + 67 MB store vs 134 MB minimum; the Pool
engine descriptor-generation time drops ~8x vs the baseline.
"""
from contextlib import ExitStack

import numpy as np

import concourse.bacc as bacc
import concourse.mybir as mybir
from concourse import library_config
from concourse.bass_utils import run_bass_kernel_spmd

# Problem shape (hardcoded per contract)
T = 16          # tables
R = 200000      # rows per table
D = 64          # embedding dim
L = 131072      # lookups per table
NCORES = 8
TPC = T // NCORES           # tables per core
N = TPC * L                 # lookups per core (262144)
ROWS_C = TPC * R            # rows per core (400000)

W = 32768                   # lookups per window
NW = N // W                 # 8 windows
WPT = NW // TPC             # windows per table (4)
BLK = 32768                 # rows per gather block (int16 index range)
NB = (R + BLK - 1) // BLK   # 7 blocks per table (each window hits one table)
BLK_ROWS = [min(BLK, R - b * BLK) for b in range(NB)]

# Per-block slot caps (multiples of 128). Uniform indices give ~5370 per
# full block (sigma ~67) and ~556 for the 3392-row tail block.
DEFAULT_CAPS = tuple([5760] * (NB - 1) + [768])

_NC_CACHE = {}


def build_nc(caps=DEFAULT_CAPS):
    caps = tuple(caps)
    if caps in _NC_CACHE:
        return _NC_CACHE[caps]
    assert len(caps) == NB and all(c % 128 == 0 for c in caps)
    S = sum(caps)                       # staging slots per window
    offs = np.cumsum([0] + list(caps))  # slot offset of each block segment
    IC = S // 16                        # idx tile columns
    # SWDGE ring holds dynamic_dma_scratch_size//16 = 1024 entries; a scatter
    # consumes num_idxs//8 + 1, so split it into ring-sized chunks.
    SCHUNK = 7936
    sbounds = list(range(0, S, SCHUNK)) + [S]
    NCH = len(sbounds) - 1              # scatter chunks per window

    # detect_race_conditions=False: CoreSim's detector cannot prove the
    # scatter chunks hit disjoint own/peer cells (data-dependent indices).
    nc = bacc.Bacc("TRN2", target_bir_lowering=False, debug=False,
                   detect_race_conditions=False)
    w_t = nc.dram_tensor("w", [ROWS_C, D], mybir.dt.float32, kind="ExternalInput")
    gi = nc.dram_tensor("gi", [NW * 128, IC], mybir.dt.int16, kind="ExternalInput")
    si = nc.dram_tensor("si", [NW * 128, IC], mybir.dt.int16, kind="ExternalInput")
    out = nc.dram_tensor("out", [N, D], mybir.dt.float32, kind="ExternalOutput")
    # window w, partition p -> output rows [w*W + p*256, w*W + (p+1)*256)
    out_v = out.ap().rearrange("(q m) d -> q (m d)", q=NW * 128)  # [1024, 256*D]

    with ExitStack() as ctx:
        git = [ctx.enter_context(nc.sbuf_tensor(f"git{k}", [128, IC], mybir.dt.int16))
               for k in range(2)]
        sit = [ctx.enter_context(nc.sbuf_tensor(f"sit{k}", [128, IC], mybir.dt.int16))
               for k in range(2)]
        staging = ctx.enter_context(
            nc.sbuf_tensor("staging", [128, (S // 128) * D], mybir.dt.float32))
        own = ctx.enter_context(
            nc.sbuf_tensor("own", [128, 128 * D], mybir.dt.float32))
        peer = ctx.enter_context(
            nc.sbuf_tensor("peer", [128, 128 * D], mybir.dt.float32))
        merged = ctx.enter_context(
            nc.sbuf_tensor("merged", [128, 256 * D], mybir.dt.bfloat16))

        igsem = [ctx.enter_context(nc.semaphore(f"igsem{k}")) for k in range(2)]
        issem = [ctx.enter_context(nc.semaphore(f"issem{k}")) for k in range(2)]
        gsem = ctx.enter_context(nc.semaphore("gsem"))   # gathers
        ssem = ctx.enter_context(nc.semaphore("ssem"))   # scatters
        zsem = ctx.enter_context(nc.semaphore("zsem"))   # own/peer zeroed
        msem = ctx.enter_context(nc.semaphore("msem"))   # merges
        stsem = ctx.enter_context(nc.semaphore("stsem"))  # output stores
        block = ctx.enter_context(nc.Block())

        stag3 = staging.ap().rearrange("p (c d) -> p c d", d=D)
        own3 = own.ap().rearrange("p (g d) -> p g d", d=D)
        peer3 = peer.ap().rearrange("p (g d) -> p g d", d=D)
        own4 = own.ap().rearrange("p (g one d) -> p g one d", one=1, d=D)
        peer4 = peer.ap().rearrange("p (g one d) -> p g one d", one=1, d=D)
        mrg4 = merged.ap().rearrange("p (g two d) -> p g two d", two=2, d=D)

        @block.sync
        def _(sync):
            for w in range(NW):
                if w >= 1:
                    # tile w-2 reused; free once scatter w-2 has drained
                    sync.wait_ge(ssem, 16 * NCH * (w - 1))
                sync.dma_start(git[w % 2][:], gi.ap()[w * 128:(w + 1) * 128, :]
                               ).then_inc(igsem[w % 2], 16)
                sync.dma_start(sit[w % 2][:], si.ap()[w * 128:(w + 1) * 128, :]
                               ).then_inc(issem[w % 2], 16)

        @block.vector
        def _(vector):
            vector.memset(own[:], 0.0).then_inc(zsem, 1)
            vector.memset(peer[:], 0.0).then_inc(zsem, 1)
            for w in range(NW):
                vector.wait_ge(ssem, 16 * NCH * (w + 1))  # scatter w drained
                vector.wait_ge(stsem, 16 * w)        # store w-1 done (merged free)
                vector.tensor_copy(mrg4[:, :, 0:1, :], own4).then_inc(msem, 1)
                vector.tensor_copy(mrg4[:, :, 1:2, :], peer4).then_inc(msem, 1)
                vector.memset(own[:], 0.0).then_inc(zsem, 1)
                vector.memset(peer[:], 0.0).then_inc(zsem, 1)

        @block.gpsimd
        def _(gpsimd):
            gpsimd.load_library(library_config.mlp)
            for w in range(NW):
                if w >= 1:
                    # store window w-1 (merged ready after its 2 merges)
                    gpsimd.wait_ge(msem, 2 * w)
                    gpsimd.dma_start(
                        out_v[(w - 1) * 128:w * 128, :], merged[:]
                    ).then_inc(stsem, 16)
                gpsimd.wait_ge(igsem[w % 2], 16 * (w // 2 + 1))  # idx tiles w loaded
                gpsimd.wait_ge(issem[w % 2], 16 * (w // 2 + 1))
                gpsimd.wait_ge(ssem, 16 * NCH * w)   # staging free
                tbase = (w // WPT) * R               # this window's table
                for b in range(NB):
                    c0, c1 = offs[b] // 128, offs[b + 1] // 128
                    gpsimd.dma_gather(
                        out_ap=stag3[:, c0:c1, :],
                        in_ap=w_t.ap()[tbase + b * BLK:
                                       tbase + b * BLK + BLK_ROWS[b], :],
                        idxs_ap=git[w % 2][:, offs[b] // 16:offs[b + 1] // 16],
                        num_idxs=caps[b], num_idxs_reg=caps[b], elem_size=D,
                        single_packet=False,
                    ).then_inc(gsem, 16)
                gpsimd.wait_ge(gsem, 16 * NB * (w + 1))  # gathers w drained
                gpsimd.wait_ge(zsem, 2 * (w + 1))        # own/peer zeroed
                for k in range(NCH):
                    s0, s1 = sbounds[k], sbounds[k + 1]
                    gpsimd.dma_scatter_add(
                        out_ap=own3,
                        in_ap=stag3[:, s0 // 128:s1 // 128, :],
                        idxs_ap=sit[w % 2][:, s0 // 16:s1 // 16],
                        num_idxs=s1 - s0, num_idxs_reg=s1 - s0, elem_size=D,
                        sbuf_tokens_per_rank=128, parity_reg=0,
                        out_ap_other=peer3, single_packet=False,
                    ).then_inc(ssem, 16)
            gpsimd.wait_ge(msem, 2 * NW)
            gpsimd.dma_start(
                out_v[(NW - 1) * 128:NW * 128, :], merged[:]
            ).then_inc(stsem, 16)
            gpsimd.wait_ge(stsem, 16 * NW)

    nc.compile()
    _NC_CACHE[caps] = nc
    return nc


def _wrap16(arr, cols):
    """[n] int16 -> [16, cols] wrap (slot j -> [j%16, j//16]), n == 16*cols."""
    return np.ascontiguousarray(arr.reshape(cols, 16).T)


def shard_inputs(indices, weights):
    """Full inputs -> per-core in_maps + caps + host patch info."""
    idx_all = indices.astype(np.int64)
    # global row = table_in_core * R + idx
    per_core = []
    counts_max = np.zeros(NB, dtype=np.int64)
    for c in range(NCORES):
        t0 = c * TPC
        rows = (idx_all[t0:t0 + TPC]
                + (np.arange(TPC, dtype=np.int64) * R)[:, None]).reshape(N)
        per_core.append(rows)
        for w in range(NW):
            win = rows[w * W:(w + 1) * W] - (w // WPT) * R
            counts_max = np.maximum(
                counts_max, np.bincount(win >> 15, minlength=NB))
    caps = tuple(int(-(-max(int(m) + 64, 128) // 128) * 128) for m in counts_max)
    if all(m <= d for m, d in zip(caps, DEFAULT_CAPS)):
        caps = DEFAULT_CAPS
    S = sum(caps)
    offs = np.cumsum([0] + list(caps))

    in_maps = []
    patches = []   # (core, window, position0 full-output row value)
    for c in range(NCORES):
        t0 = c * TPC
        w_c = np.ascontiguousarray(weights[t0:t0 + TPC]).reshape(ROWS_C, D)
        rows = per_core[c]
        gi = np.empty((NW, 128, S // 16), dtype=np.int16)
        si = np.empty((NW, 128, S // 16), dtype=np.int16)
        for w in range(NW):
            win = rows[w * W:(w + 1) * W] - (w // WPT) * R
            blk = (win >> 15).astype(np.int64)
            order = np.argsort(blk, kind="stable")
            counts = np.bincount(blk, minlength=NB)
            g_slots = np.zeros(S, dtype=np.int16)
            s_slots = np.zeros(S, dtype=np.int16)
            pos = 0
            for b in range(NB):
                nb_ = int(counts[b])
                sel = order[pos:pos + nb_]
                pos += nb_
                g_slots[offs[b]:offs[b] + nb_] = (win[sel] - (b << 15)).astype(np.int16)
                s_slots[offs[b]:offs[b] + nb_] = (
                    ((sel & 255) << 7) | (sel >> 8)).astype(np.int16)
                # padding slots keep g=0 (reads block base row), s=0 (trash)
            g16 = _wrap16(g_slots, S // 16)
            s16 = _wrap16(s_slots, S // 16)
            gi[w] = np.tile(g16, (8, 1))
            si[w] = np.tile(s16, (8, 1))
        in_maps.append({
            "w": w_c,
            "gi": gi.reshape(NW * 128, S // 16),
            "si": si.reshape(NW * 128, S // 16),
        })
        for w in range(NW):
            patches.append((c, w, w_c[rows[w * W]]))
    return caps, in_maps, patches


def kernel(indices: np.ndarray, weights: np.ndarray, **run_kwargs) -> np.ndarray:
    indices = np.asarray(indices, dtype=np.int32)
    weights = np.asarray(weights, dtype=np.float32)
    assert indices.shape == (T, L) and weights.shape == (T, R, D)

    caps, in_maps, patches = shard_inputs(indices, weights)
    nc = build_nc(caps)
    res = run_bass_kernel_spmd(nc, in_maps, core_ids=list(range(NCORES)),
                               **run_kwargs)
    out = np.concatenate([r["out"] for r in res.results], axis=0)
    # window position 0 is the scatter trash target; restore exact rows
    for c, w, row in patches:
        out[c * N + w * W] = row
    kernel.last_results = res
    return out


# revision 16
# speedup vs baseline: 2.1218x; 2.1218x over previous
"""GroupedEmbedding lookup kernel for 8 Trainium2 NeuronCores.

Sharding: table-wise, 2 tables per core. Each core holds a [2*R, D] weight
slab and processes the matching 262144 lookups; its output is a contiguous
block of the final [T*L, D] output.

Device kernel v2 (bulk-descriptor gather/scatter via GPSIMD mlp library):

The baseline used one indirect_dma_start per 128 lookups; SWDGE descriptor
generation costs ~1 us *per instruction* (994 ns fixed + 0.34 ns/descriptor),
so 2048 instructions serialized on the Pool engine at ~2.9 ms. dma_gather
(InstDMAGatherAnt) generates thousands of descriptors in ONE instruction,
but takes int16 indices (<= 32767) while the per-core slab has 400000 rows.

Per 32768-lookup window:
  1. Host buckets the window's lookups by 32768-row block (13 blocks) and
     emits per-block int16 local indices (padded to a fixed cap with idx 0)
     plus int16 scatter codes that undo the bucket permutation.
  2. 13x dma_gather: block rows -> staging SBUF (bucket order).
  3. 1x dma_scatter_add (SBUF parity mode) staging -> own/peer buffers:
     scatter code v = (i & 255)*128 + (i >> 8) routes window position i to
     own/peer[partition i>>8, group (i&255)>>1]; padding goes to v=0 (trash;
     position 0 of each window is patched on the host afterwards).
     own/peer are pre-zeroed by DVE (scatter is an ADD).
  4. DVE interleaves own/peer -> merged bf16 [128, 256*D] (position-ordered,
     contiguous per partition).
  5. gpsimd cast-store (bf16 -> f32) merged -> output window (64 KB/partition
     contiguous runs).

HBM traffic/core ~= 73 MB gather + 67 MB store vs 134 MB minimum; the Pool
engine descriptor-generation time drops ~8x vs the baseline.
"""
from contextlib import ExitStack

import numpy as np

import concourse.bacc as bacc
import concourse.mybir as mybir
from concourse import library_config
from concourse.bass_utils import run_bass_kernel_spmd

# Problem shape (hardcoded per contract)
T = 16          # tables
R = 200000      # rows per table
D = 64          # embedding dim
L = 131072      # lookups per table
NCORES = 8
TPC = T // NCORES           # tables per core
N = TPC * L                 # lookups per core (262144)
ROWS_C = TPC * R            # rows per core (400000)

W = 32768                   # lookups per window
NW = N // W                 # 8 windows
WPT = NW // TPC             # windows per table (4)
BLK = 32768                 # rows per gather block (int16 index range)
NB = (R + BLK - 1) // BLK   # 7 blocks per table (each window hits one table)
BLK_ROWS = [min(BLK, R - b * BLK) for b in range(NB)]

# Per-block slot caps (multiples of 128). Uniform indices give ~5370 per
# full block (sigma ~67) and ~556 for the 3392-row tail block.
DEFAULT_CAPS = tuple([5760] * (NB - 1) + [768])

_NC_CACHE = {}


def build_nc(caps=DEFAULT_CAPS):
    caps = tuple(caps)
    if caps in _NC_CACHE:
        return _NC_CACHE[caps]
    assert len(caps) == NB and all(c % 128 == 0 for c in caps)
    S = sum(caps)                       # staging slots per window
    offs = np.cumsum([0] + list(caps))  # slot offset of each block segment
    IC = S // 16                        # idx tile columns
    # SWDGE ring holds dynamic_dma_scratch_size//16 = 1024 entries; a scatter
    # consumes num_idxs//8 + 1, so split it into ring-sized chunks.
    SCHUNK = 7936
    sbounds = list(range(0, S, SCHUNK)) + [S]
    NCH = len(sbounds) - 1              # scatter chunks per window

    # detect_race_conditions=False: CoreSim's detector cannot prove the
    # scatter chunks hit disjoint own/peer cells (data-dependent indices).
    nc = bacc.Bacc("TRN2", target_bir_lowering=False, debug=False,
                   detect_race_conditions=False, num_swdge_queues=4)
    w_t = nc.dram_tensor("w", [ROWS_C, D], mybir.dt.float32, kind="ExternalInput")
    gi = nc.dram_tensor("gi", [NW * 128, IC], mybir.dt.int16, kind="ExternalInput")
    si = nc.dram_tensor("si", [NW * 128, IC], mybir.dt.int16, kind="ExternalInput")
    out = nc.dram_tensor("out", [N, D], mybir.dt.float32, kind="ExternalOutput")
    # window w, partition p -> output rows [w*W + p*256, w*W + (p+1)*256)
    out_v = out.ap().rearrange("(q m) d -> q (m d)", q=NW * 128)  # [1024, 256*D]

    with ExitStack() as ctx:
        git = [ctx.enter_context(nc.sbuf_tensor(f"git{k}", [128, IC], mybir.dt.int16))
               for k in range(2)]
        sit = [ctx.enter_context(nc.sbuf_tensor(f"sit{k}", [128, IC], mybir.dt.int16))
               for k in range(2)]
        staging = ctx.enter_context(
            nc.sbuf_tensor("staging", [128, (S // 128) * D], mybir.dt.float32))
        own = ctx.enter_context(
            nc.sbuf_tensor("own", [128, 128 * D], mybir.dt.float32))
        peer = ctx.enter_context(
            nc.sbuf_tensor("peer", [128, 128 * D], mybir.dt.float32))
        merged = ctx.enter_context(
            nc.sbuf_tensor("merged", [128, 256 * D], mybir.dt.bfloat16))

        igsem = [ctx.enter_context(nc.semaphore(f"igsem{k}")) for k in range(2)]
        issem = [ctx.enter_context(nc.semaphore(f"issem{k}")) for k in range(2)]
        gsemq = [ctx.enter_context(nc.semaphore(f"gsemq{q}")) for q in range(4)]
        ssemq = [ctx.enter_context(nc.semaphore(f"ssemq{q}")) for q in range(4)]
        zsem = ctx.enter_context(nc.semaphore("zsem"))   # own/peer zeroed
        msem = ctx.enter_context(nc.semaphore("msem"))   # merges
        stsem = ctx.enter_context(nc.semaphore("stsem"))  # output stores
        block = ctx.enter_context(nc.Block())

        stag3 = staging.ap().rearrange("p (c d) -> p c d", d=D)
        own3 = own.ap().rearrange("p (g d) -> p g d", d=D)
        peer3 = peer.ap().rearrange("p (g d) -> p g d", d=D)
        own4 = own.ap().rearrange("p (g one d) -> p g one d", one=1, d=D)
        peer4 = peer.ap().rearrange("p (g one d) -> p g one d", one=1, d=D)
        mrg4 = merged.ap().rearrange("p (g two d) -> p g two d", two=2, d=D)

        # queue assignment: round-robin within each window (deterministic)
        gq = {(w, b): (w * NB + b) % 4 for w in range(NW) for b in range(NB)}
        sq = {(w, k): (w * NCH + k) % 4 for w in range(NW) for k in range(NCH)}
        # cumulative per-queue counts AFTER window w's ops
        gcum = [[0] * 4]
        scum = [[0] * 4]
        for w in range(NW):
            g = gcum[-1][:]
            for b in range(NB):
                g[gq[(w, b)]] += 1
            gcum.append(g)
            sc = scum[-1][:]
            for k in range(NCH):
                sc[sq[(w, k)]] += 1
            scum.append(sc)

        def wait_scatters(eng, w_done):
            # wait until all scatter chunks of windows < w_done drained
            for q in range(4):
                if scum[w_done][q]:
                    eng.wait_ge(ssemq[q], 16 * scum[w_done][q])

        def wait_gathers(eng, w_done):
            for q in range(4):
                if gcum[w_done][q]:
                    eng.wait_ge(gsemq[q], 16 * gcum[w_done][q])

        @block.sync
        def _(sync):
            for w in range(NW):
                if w >= 1:
                    # tile w-2 reused; free once scatter w-2 has drained
                    wait_scatters(sync, w - 1)
                sync.dma_start(git[w % 2][:], gi.ap()[w * 128:(w + 1) * 128, :]
                               ).then_inc(igsem[w % 2], 16)
                sync.dma_start(sit[w % 2][:], si.ap()[w * 128:(w + 1) * 128, :]
                               ).then_inc(issem[w % 2], 16)

        @block.vector
        def _(vector):
            vector.memset(own[:], 0.0).then_inc(zsem, 1)
            vector.memset(peer[:], 0.0).then_inc(zsem, 1)
            for w in range(NW):
                wait_scatters(vector, w + 1)         # scatter w drained
                vector.wait_ge(stsem, 16 * w)        # store w-1 done (merged free)
                vector.tensor_copy(mrg4[:, :, 0:1, :], own4).then_inc(msem, 1)
                vector.tensor_copy(mrg4[:, :, 1:2, :], peer4).then_inc(msem, 1)
                vector.memset(own[:], 0.0).then_inc(zsem, 1)
                vector.memset(peer[:], 0.0).then_inc(zsem, 1)

        @block.gpsimd
        def _(gpsimd):
            gpsimd.load_library(library_config.mlp)
            for w in range(NW):
                if w >= 1:
                    # store window w-1 (merged ready after its 2 merges)
                    gpsimd.wait_ge(msem, 2 * w)
                    gpsimd.dma_start(
                        out_v[(w - 1) * 128:w * 128, :], merged[:]
                    ).then_inc(stsem, 16)
                gpsimd.wait_ge(igsem[w % 2], 16 * (w // 2 + 1))  # idx tiles w loaded
                gpsimd.wait_ge(issem[w % 2], 16 * (w // 2 + 1))
                wait_scatters(gpsimd, w)             # staging free
                tbase = (w // WPT) * R               # this window's table
                for b in range(NB):
                    c0, c1 = offs[b] // 128, offs[b + 1] // 128
                    gpsimd.dma_gather(
                        out_ap=stag3[:, c0:c1, :],
                        in_ap=w_t.ap()[tbase + b * BLK:
                                       tbase + b * BLK + BLK_ROWS[b], :],
                        idxs_ap=git[w % 2][:, offs[b] // 16:offs[b + 1] // 16],
                        num_idxs=caps[b], num_idxs_reg=caps[b], elem_size=D,
                        single_packet=False, queue_num=gq[(w, b)],
                    ).then_inc(gsemq[gq[(w, b)]], 16)
                wait_gathers(gpsimd, w + 1)              # gathers w drained
                gpsimd.wait_ge(zsem, 2 * (w + 1))        # own/peer zeroed
                for k in range(NCH):
                    s0, s1 = sbounds[k], sbounds[k + 1]
                    gpsimd.dma_scatter_add(
                        out_ap=own3,
                        in_ap=stag3[:, s0 // 128:s1 // 128, :],
                        idxs_ap=sit[w % 2][:, s0 // 16:s1 // 16],
                        num_idxs=s1 - s0, num_idxs_reg=s1 - s0, elem_size=D,
                        sbuf_tokens_per_rank=128, parity_reg=0,
                        out_ap_other=peer3, single_packet=False,
                        queue_num=sq[(w, k)],
                    ).then_inc(ssemq[sq[(w, k)]], 16)
            gpsimd.wait_ge(msem, 2 * NW)
            gpsimd.dma_start(
                out_v[(NW - 1) * 128:NW * 128, :], merged[:]
            ).then_inc(stsem, 16)
            gpsimd.wait_ge(stsem, 16 * NW)

    nc.compile()
    _NC_CACHE[caps] = nc
    return nc


def _wrap16(arr, cols):
    """[n] int16 -> [16, cols] wrap (slot j -> [j%16, j//16]), n == 16*cols."""
    return np.ascontiguousarray(arr.reshape(cols, 16).T)


def shard_inputs(indices, weights):
    """Full inputs -> per-core in_maps + caps + host patch info."""
    idx_all = indices.astype(np.int64)
    # global row = table_in_core * R + idx
    per_core = []
    counts_max = np.zeros(NB, dtype=np.int64)
    for c in range(NCORES):
        t0 = c * TPC
        rows = (idx_all[t0:t0 + TPC]
                + (np.arange(TPC, dtype=np.int64) * R)[:, None]).reshape(N)
        per_core.append(rows)
        for w in range(NW):
            win = rows[w * W:(w + 1) * W] - (w // WPT) * R
            counts_max = np.maximum(
                counts_max, np.bincount(win >> 15, minlength=NB))
    caps = tuple(int(-(-max(int(m) + 64, 128) // 128) * 128) for m in counts_max)
    if all(m <= d for m, d in zip(caps, DEFAULT_CAPS)):
        caps = DEFAULT_CAPS
    S = sum(caps)
    offs = np.cumsum([0] + list(caps))

    in_maps = []
    patches = []   # (core, window, position0 full-output row value)
    for c in range(NCORES):
        t0 = c * TPC
        w_c = np.ascontiguousarray(weights[t0:t0 + TPC]).reshape(ROWS_C, D)
        rows = per_core[c]
        gi = np.empty((NW, 128, S // 16), dtype=np.int16)
        si = np.empty((NW, 128, S // 16), dtype=np.int16)
        for w in range(NW):
            win = rows[w * W:(w + 1) * W] - (w // WPT) * R
            blk = (win >> 15).astype(np.int64)
            order = np.argsort(blk, kind="stable")
            counts = np.bincount(blk, minlength=NB)
            g_slots = np.zeros(S, dtype=np.int16)
            s_slots = np.zeros(S, dtype=np.int16)
            pos = 0
            for b in range(NB):
                nb_ = int(counts[b])
                sel = order[pos:pos + nb_]
                pos += nb_
                g_slots[offs[b]:offs[b] + nb_] = (win[sel] - (b << 15)).astype(np.int16)
                s_slots[offs[b]:offs[b] + nb_] = (
                    ((sel & 255) << 7) | (sel >> 8)).astype(np.int16)
                # padding slots keep g=0 (reads block base row), s=0 (trash)
            g16 = _wrap16(g_slots, S // 16)
            s16 = _wrap16(s_slots, S // 16)
            gi[w] = np.tile(g16, (8, 1))
            si[w] = np.tile(s16, (8, 1))
        in_maps.append({
            "w": w_c,
            "gi": gi.reshape(NW * 128, S // 16),
            "si": si.reshape(NW * 128, S // 16),
        })
        for w in range(NW):
            patches.append((c, w, w_c[rows[w * W]]))
    return caps, in_maps, patches


def kernel(indices: np.ndarray, weights: np.ndarray, **run_kwargs) -> np.ndarray:
    indices = np.asarray(indices, dtype=np.int32)
    weights = np.asarray(weights, dtype=np.float32)
    assert indices.shape == (T, L) and weights.shape == (T, R, D)

    caps, in_maps, patches = shard_inputs(indices, weights)
    nc = build_nc(caps)
    res = run_bass_kernel_spmd(nc, in_maps, core_ids=list(range(NCORES)),
                               **run_kwargs)
    out = np.concatenate([r["out"] for r in res.results], axis=0)
    # window position 0 is the scatter trash target; restore exact rows
    for c, w, row in patches:
        out[c * N + w * W] = row
    kernel.last_results = res
    return out
